# revision 9
# baseline (speedup 1.0000x reference)
"""Trainium2 Bass kernel for the BreakthroughSNN encoder problem (v2).

Per (b, t, s, d):
    out = w0*rate + w1*temporal + w2*pop + w3*phase
    rate    = 1[rate_rand < clip(sig*0.9+0.05+0.1*noise, 0, 1)]
    temporal= 1[floor(sig*(T-1)) == t]
    pop     = mean_n 1[pop_rand < sigmoid(emb @ pop_W + b)]
    phase   = 1[sin(freq_d*t_k + sig*2pi) > 0.5],  sig = sigmoid(emb)

v2 strategy (vs v1 = all-f32):
  - Host quantizes the uniform randoms: pop_rand -> u8, rate_rand -> u16.
    Comparisons against a quantized uniform sample only flip when the
    threshold falls inside one quantization cell (p~2^-9 / 2^-17), far
    inside the 2e-2 rel-err budget.  HBM traffic drops 47MB -> ~16MB/core.
  - pop_rand u8 is cast to bf16 in-flight by SWDGE DMA; the big [128,4096]
    compare runs on DVE in bf16 (2x mode).
  - popcount over n and the weighted combine run on the PE as accumulating
    scaled-identity matmuls into PSUM; ACT casts psum -> u8 output
    (k = pop + 8*rate + 8*temporal + 4*sgn + 4 is an exact small int;
    host multiplies by w0/8 = 1/32).
  - phase: theta' - pi assembled in PSUM from bf16 split terms
    (phi1+phi2 per-token idents + 3 stacked host rows), folded into
    (-pi,pi] with an ACT Sign + two split -pi idents, then ACT Sin.
  - temporal one-hot via c_t = 1[st < t+0.5] and temp = c_t - c_{t-1}.
"""

import os
import sys

for _p in ("/opt/trn_rl_repo", os.path.expanduser("~/.axon_site/_ro/trn_rl_repo")):
    if os.path.isdir(_p) and _p not in sys.path:
        sys.path.insert(0, _p)

import numpy as np
import ml_dtypes

import concourse.bacc as bacc
import concourse.mybir as mybir
import concourse.tile as tile
from concourse.bass_utils import run_bass_kernel_spmd

Alu = mybir.AluOpType
Act = mybir.ActivationFunctionType
F32 = mybir.dt.float32
BF16 = mybir.dt.bfloat16
U8 = mybir.dt.uint8
U16 = mybir.dt.uint16

TWO_PI = 2.0 * np.pi

B, T, S, D, N = 4, 16, 256, 512, 8
NCORES = 8
NTOK = B * S                 # 1024 tokens
TOK = NTOK // NCORES         # 128 tokens per core (= partition dim)
DN = D * N                   # 4096
TD = T * D                   # 8192

BF = ml_dtypes.bfloat16


def _bf16_split(x, n):
    """Split float64 array into n bf16 terms summing (in f32) to ~x."""
    parts = []
    rem = x.astype(np.float64)
    for _ in range(n):
        p = rem.astype(BF)
        parts.append(p)
        rem = rem - p.astype(np.float64)
    return parts


def _build_program(coefs, has_bias, uniform):
    """coefs = (a_pop, a_r, a_c, a_s, bias0) baked into identity lhsTs."""
    from contextlib import ExitStack

    a_pop, a_r, a_c, a_s, bias0 = coefs
    nk = D // 128 + (1 if has_bias else 0)   # K-chunks of the pop matmul
    kdim = nk * 128

    nc = bacc.Bacc("TRN2", target_bir_lowering=False, debug=False,
                   num_devices=NCORES)

    emb = nc.dram_tensor("emb", [TOK, D], F32, kind="ExternalInput")
    embT = nc.dram_tensor("embT", [kdim, TOK], BF16, kind="ExternalInput")
    noise = nc.dram_tensor("noise", [TOK, D], F32, kind="ExternalInput")
    rr = nc.dram_tensor("rr", [TOK, TD], U16, kind="ExternalInput")
    pr = nc.dram_tensor("pr", [T, TOK, DN], U8, kind="ExternalInput")
    Wd = nc.dram_tensor("W", [kdim, DN], BF16, kind="ExternalInput")
    # idents blob: 8 scaled [128,128] idents + [3,128] ones rows
    idd = nc.dram_tensor("idents", [8 * 128 + 3, 128], BF16,
                         kind="ExternalInput")
    srd = nc.dram_tensor("srows", [3, TD], BF16, kind="ExternalInput")
    out_dt = U8 if uniform else BF16
    outd = nc.dram_tensor("out", [TOK, TD], out_dt, kind="ExternalOutput")

    with tile.TileContext(nc) as tc, ExitStack() as ctx:
        const = ctx.enter_context(tc.tile_pool(name="const", bufs=1))
        wp = ctx.enter_context(tc.tile_pool(name="wp", bufs=4))
        prp = ctx.enter_context(tc.tile_pool(name="prp", bufs=4))
        spkp = ctx.enter_context(tc.tile_pool(name="spkp", bufs=2))
        wvp = ctx.enter_context(tc.tile_pool(name="wvp", bufs=2))
        pm = ctx.enter_context(tc.tile_pool(name="pm", bufs=1, space="PSUM"))
        pw = ctx.enter_context(tc.tile_pool(name="pw", bufs=2, space="PSUM"))
        pc = ctx.enter_context(tc.tile_pool(name="pc", bufs=2, space="PSUM"))

        # ---- one-time loads ----
        # smalls first so ACT/DVE precompute can start immediately;
        # rr chunked on the scalar HWDGE queue; W (4MB) last on sync.
        emb_sb = const.tile([TOK, D], F32)
        nc.sync.dma_start(emb_sb[:], emb[:])
        noise_sb = const.tile([TOK, D], F32)
        nc.sync.dma_start(noise_sb[:], noise[:])
        idt = []
        for i in range(8):
            it = const.tile([128, 128], BF16, tag=f"id{i}")
            nc.sync.dma_start(it[:], idd[i * 128:(i + 1) * 128, :])
            idt.append(it[:])
        I_pop, I_r, I_c, I_cm, I_s, I_1, I_p1, I_p2 = idt
        ones3t = const.tile([3, 128], BF16)
        nc.sync.dma_start(ones3t[:], idd[8 * 128:8 * 128 + 3, :])
        ones3 = ones3t[:]
        lhsT = []
        for k in range(nk):
            lt = const.tile([128, TOK], BF16, tag=f"lhsT{k}")
            nc.sync.dma_start(lt[:], embT[k * 128:(k + 1) * 128, :])
            lhsT.append(lt)
        rr_sb = const.tile([TOK, TD], U16)
        for j in range(4):
            nc.scalar.dma_start(rr_sb[:, j * 4 * D:(j + 1) * 4 * D],
                                rr[:, j * 4 * D:(j + 1) * 4 * D])
        wts = []
        for k in range(nk):
            wt = wp.tile([128, DN], BF16, tag="w")
            nc.sync.dma_start(wt[:], Wd[k * 128:(k + 1) * 128, :])
            wts.append(wt)

        # ---- per-token precompute ----
        sig = const.tile([TOK, D], F32)
        nc.scalar.activation(sig[:], emb_sb[:], Act.Sigmoid)

        rates64 = const.tile([TOK, D], F32)
        tmp = const.tile([TOK, D], F32)
        nc.vector.tensor_scalar(tmp[:], sig[:], 0.9, 0.05, Alu.mult, Alu.add)
        nc.vector.scalar_tensor_tensor(tmp[:], noise_sb[:], 0.1, tmp[:],
                                       Alu.mult, Alu.add)
        nc.vector.tensor_scalar(tmp[:], tmp[:], 0.0, 1.0, Alu.max, Alu.min)
        nc.vector.tensor_scalar(rates64[:], tmp[:], 65536.0, None, Alu.mult)

        # st = floor(sig*(T-1)) via RNE(+-2^23) and round-down correction
        st = const.tile([TOK, D], F32)
        x15 = const.tile([TOK, D], F32)
        nc.vector.tensor_scalar(x15[:], sig[:], float(T - 1), None, Alu.mult)
        rnd = const.tile([TOK, D], F32)
        nc.vector.tensor_scalar(rnd[:], x15[:], 8388608.0, 8388608.0,
                                Alu.add, Alu.subtract)
        gtt = const.tile([TOK, D], F32)
        nc.vector.tensor_tensor(gtt[:], rnd[:], x15[:], Alu.is_gt)
        nc.vector.tensor_tensor(st[:], rnd[:], gtt[:], Alu.subtract)

        # phi split: phi = sig*2pi ; phi1 = bf16(phi) ; phi2 = bf16(phi-phi1)
        phi = const.tile([TOK, D], F32)
        nc.vector.tensor_scalar(phi[:], sig[:], TWO_PI, None, Alu.mult)
        phi1 = const.tile([TOK, D], BF16)
        nc.scalar.activation(phi1[:], phi[:], Act.Copy)
        phi2 = const.tile([TOK, D], BF16)
        nc.vector.tensor_tensor(phi2[:], phi[:], phi1[:], Alu.subtract)

        # ---- rate bits (u16 vs f32, 1x) ----
        rbit = const.tile([TOK, TD], BF16)
        for t in range(T):
            sl = slice(t * D, (t + 1) * D)
            nc.vector.tensor_tensor(rbit[:, sl], rr_sb[:, sl], rates64[:],
                                    Alu.is_lt)

        # ---- G = rbit + (a_c/a_r)*(c_t - c_{t-1}) + (a_s/a_r)*sgn ----
        # (batched; combine then needs only one a_r*I @ G_t matmul per t)
        G = const.tile([TOK, TD], BF16)
        r_c = a_c / a_r
        r_s = a_s / a_r

        def emit_g(j):
            sl = slice(j * 4 * D, (j + 1) * 4 * D)
            slD = slice(j * 4 * D + D, (j + 1) * 4 * D + D)
            if abs(r_c - 1.0) < 1e-12:
                # gpsimd supports plain tensor_tensor; STT only on DVE
                nc.gpsimd.tensor_tensor(G[:, sl], cz[:, slD], rbit[:, sl],
                                        Alu.add)
            else:
                nc.vector.scalar_tensor_tensor(G[:, sl], cz[:, slD], r_c,
                                               rbit[:, sl], Alu.mult, Alu.add)
            nc.vector.scalar_tensor_tensor(G[:, sl], cz[:, sl], -r_c,
                                           G[:, sl], Alu.mult, Alu.add)
            nc.vector.scalar_tensor_tensor(G[:, sl], sgn[:, sl], r_s,
                                           G[:, sl], Alu.mult, Alu.add)

        # ---- temporal cumulative bits: cz[:, (1+t)*D:] = 1[st < t+0.5] ----
        cz = const.tile([TOK, (T + 1) * D], BF16)
        nc.vector.memset(cz[:, 0:D], 0.0)
        for t in range(T):
            nc.vector.tensor_scalar(cz[:, (1 + t) * D:(2 + t) * D], st[:],
                                    t + 0.5, None, Alu.is_lt)

        # ---- pop linear: presp = emb @ W, sigmoid, *256 -> bf16 ----
        prt256 = const.tile([TOK, DN], BF16)
        for h in range(2):
            ps = pm.tile([128, DN // 2], F32, tag="pm")
            for k in range(nk):
                for j in range(4):
                    sl = slice(j * 512, (j + 1) * 512)
                    nc.tensor.matmul(ps[:, sl], lhsT[k][:],
                                     wts[k][:, h * (DN // 2) + j * 512:
                                            h * (DN // 2) + (j + 1) * 512],
                                     start=(k == 0), stop=(k == nk - 1))
            nc.scalar.activation(prt256[:, h * (DN // 2):(h + 1) * (DN // 2)],
                                 ps[:], Act.Sigmoid)
        # *256 in place: bf16 exponent shift, exact
        nc.vector.tensor_scalar(prt256[:], prt256[:], 256.0, None, Alu.mult)

        # ---- waves: per t-step chunk of 512 cols ----
        # psum q = phi + S'_t  (S' = f32(t*f) - 2pi*k0 - pi, 3 bf16 rows)
        # sg = Sign(q); q += -pi_hi*sg + -pi_lo*sg  -> q in (-pi, pi]
        # wv = Sin(q); sgn_t = Sign(wv - 0.5)  in {-1, 0, 1}
        sgn = const.tile([TOK, TD], BF16)
        negh = const.tile([TOK, 1], F32)
        nc.vector.memset(negh[:], -0.5)

        def emit_wave(t):
            qs = pw.tile([128, D], F32, tag="pw")
            sl = slice(t * D, (t + 1) * D)
            sr = wvp.tile([3, D], BF16, tag="sr")
            nc.sync.dma_start(sr[:], srd[:, sl])
            nc.tensor.matmul(qs[:], I_1, phi1[:], start=True, stop=False)
            nc.tensor.matmul(qs[:], I_1, phi2[:], start=False, stop=False)
            nc.tensor.matmul(qs[:], ones3, sr[:], start=False,
                             stop=True)
            sg = wvp.tile([TOK, D], BF16, tag="sg")
            nc.scalar.activation(sg[:], qs[:], Act.Sign)
            nc.tensor.matmul(qs[:], I_p1, sg[:], start=False, stop=False,
                             skip_group_check=True)
            nc.tensor.matmul(qs[:], I_p2, sg[:], start=False, stop=True,
                             skip_group_check=True)
            wv = wvp.tile([TOK, D], F32, tag="wv")
            nc.scalar.activation(wv[:], qs[:], Act.Sin)
            nc.scalar.activation(sgn[:, sl], wv[:], Act.Sign, bias=negh[:])

        # ---- output (double-buffered, flushed every 4 t) ----
        outp = ctx.enter_context(tc.tile_pool(name="outp", bufs=2))

        # ---- t-loop ----
        for t in range(T):
            emit_wave(t)
            if t % 4 == 3:
                emit_g(t // 4)

        out_sb = None
        for t in range(T):
            if t % 4 == 0:
                out_sb = outp.tile([TOK, 4 * D], out_dt, tag="out")
            pt = prp.tile([TOK, DN], BF16, tag="pt")
            nc.gpsimd.dma_start(pt[:], pr[t])          # SWDGE u8 -> bf16 cast
            spk = spkp.tile([TOK, DN], BF16, tag="spk")
            nc.vector.tensor_tensor(spk[:], pt[:], prt256[:], Alu.is_lt)

            cs = pc.tile([128, D], F32, tag="pc")
            for n in range(8):
                nc.tensor.matmul(cs[:], I_pop, spk[:, n * D:(n + 1) * D],
                                 start=(n == 0), stop=False)
            sl = slice(t * D, (t + 1) * D)
            nc.tensor.matmul(cs[:], I_r, G[:, sl], start=False, stop=True)
            osl = slice((t % 4) * D, (t % 4 + 1) * D)
            nc.scalar.activation(out_sb[:, osl], cs[:], Act.Copy, bias=bias0,
                                 scale=1.0)
            if t % 4 == 3:
                nc.scalar.dma_start(outd[:, (t - 3) * D:(t + 1) * D], out_sb[:])

    nc.compile()
    return nc


def _prepare_inputs(embeddings, pop_W, pop_b, freq_bands, enc_weights,
                    rate_noise, rate_rand, pop_rand):
    """Host-side sharding + layout/dtype transforms -> per-core in_maps."""
    e = np.exp(enc_weights.astype(np.float64)
               - enc_weights.astype(np.float64).max())
    w = e / e.sum()
    w0, w1, w2, w3 = [float(x) for x in w]
    uniform = abs(w1 - w0) < 1e-12 and abs(w3 - w0) < 1e-12 \
        and abs(w2 - w0) < 1e-12

    has_bias = bool(np.any(pop_b != 0))
    kdim = D + (128 if has_bias else 0)

    emb_f = np.ascontiguousarray(embeddings.reshape(NTOK, D))
    noise_f = np.ascontiguousarray(rate_noise.reshape(NTOK, D))
    # rate_rand [B,T,S,D] -> [BS, T*D] u16
    rr_f = rate_rand.transpose(0, 2, 1, 3).reshape(NTOK, TD)
    rr_u16 = np.minimum(np.floor(rr_f.astype(np.float64) * 65536.0),
                        65535).astype(np.uint16)
    # pop_rand [B,T,S,D,N] -> [BS, T, N*D] u8 (n-major feature axis)
    pr_f = pop_rand.transpose(0, 2, 1, 4, 3).reshape(NTOK, T, DN)
    pr_u8 = np.minimum(np.floor(pr_f.astype(np.float64) * 256.0),
                       255).astype(np.uint8)
    # pop_W columns to n-major: W2[k, n*D+d] = pop_W[k, d*N+n]
    W2 = pop_W.reshape(D, D, N).transpose(0, 2, 1).reshape(D, DN)
    if has_bias:
        b_nm = pop_b.reshape(D, N).T.reshape(1, DN)
        W2 = np.vstack([W2, b_nm, np.zeros((127, DN), np.float32)])
    W2 = np.ascontiguousarray(W2.astype(BF))

    # S rows: match jnp.linspace bit-exactly, tfc = f32(t*f) as jax does
    import jax
    import jax.numpy as jnp
    with jax.default_device(jax.devices("cpu")[0]):
        t_lin = np.asarray(jnp.linspace(0.0, TWO_PI, T)).astype(np.float64)
    tfc = (t_lin[:, None] * freq_bands.astype(np.float64)[None, :]
           ).astype(np.float32)                       # [T, D] f32 as jax
    k0 = np.round(tfc.astype(np.float64) / TWO_PI)
    Sp = tfc.astype(np.float64) - TWO_PI * k0 - np.pi   # in (-2pi, 0]
    s1, s2, s3 = _bf16_split(Sp.reshape(1, TD), 3)
    srows = np.ascontiguousarray(np.vstack([s1, s2, s3]))

    # coefficient idents
    if uniform:
        a_pop, a_r, a_c, a_s, bias0 = 1.0, 8.0, 8.0, 4.0, 4.0
    else:
        a_pop, a_r, a_c, a_s, bias0 = w2 / 8, w0, w1, w3 / 2, w3 / 2
    p1 = float(np.float64(np.pi).astype(BF))            # bf16(pi), exact rep
    p2 = float((np.float64(np.pi) - p1).astype(BF))
    I = np.eye(128, dtype=np.float64)
    blob = np.vstack([a_pop * I, a_r * I, a_c * I, -a_c * I, a_s * I,
                      1.0 * I, -p1 * I, -p2 * I,
                      np.ones((3, 128), np.float64)]).astype(BF)
    blob = np.ascontiguousarray(blob)

    in_maps = []
    for c in range(NCORES):
        t0, t1 = c * TOK, (c + 1) * TOK
        eT = emb_f[t0:t1].T
        if has_bias:
            eT = np.vstack([eT, np.ones((1, TOK), np.float32),
                            np.zeros((127, TOK), np.float32)])
        in_maps.append({
            "emb": emb_f[t0:t1],
            "embT": np.ascontiguousarray(eT.astype(BF)),
            "noise": noise_f[t0:t1],
            "rr": np.ascontiguousarray(rr_u16[t0:t1]),
            "pr": np.ascontiguousarray(pr_u8[t0:t1].transpose(1, 0, 2)),
            "W": W2,
            "idents": blob,
            "srows": srows,
        })
    return in_maps, (a_pop, a_r, a_c, a_s, bias0), has_bias, uniform, w0


_cache = {}


def kernel(embeddings, pop_W, pop_b, freq_bands, enc_weights,
           rate_noise, rate_rand, pop_rand, _want_trace=False):
    in_maps, coefs, has_bias, uniform, w0 = _prepare_inputs(
        embeddings, pop_W, pop_b, freq_bands, enc_weights,
        rate_noise, rate_rand, pop_rand)

    key = (coefs, has_bias, uniform)
    if key not in _cache:
        _cache[key] = _build_program(coefs, has_bias, uniform)
    nc = _cache[key]

    res = run_bass_kernel_spmd(nc, in_maps, core_ids=list(range(NCORES)),
                               trace=_want_trace)

    # out per core: [TOK, T*D] -> full [B, T, S, D]
    full = np.empty((NTOK, T, D), np.float32)
    scale = np.float32(w0 / 8.0) if uniform else np.float32(1.0)
    for c in range(NCORES):
        o = res.results[c]["out"].astype(np.float32).reshape(TOK, T, D)
        full[c * TOK:(c + 1) * TOK] = o
    if uniform:
        full *= scale
    out = full.reshape(B, S, T, D).transpose(0, 2, 1, 3)
    out = np.ascontiguousarray(out)
    if _want_trace:
        kernel._last_trace = res
    return out


# revision 10
# speedup vs baseline: 1.1606x; 1.1606x over previous
"""Trainium2 Bass kernel for the BreakthroughSNN encoder problem (v4).

Per (b, t, s, d):
    out = w0*rate + w1*temporal + w2*pop + w3*phase
    rate    = 1[rate_rand < clip(sig*0.9+0.05+0.1*noise, 0, 1)]
    temporal= 1[floor(sig*(T-1)) == t]
    pop     = mean_n 1[pop_rand < sigmoid(emb @ pop_W + b)]
    phase   = 1[sin(freq_d*t_k + sig*2pi) > 0.5],  sig = sigmoid(emb)

Strategy:
  - Host quantizes the uniform randoms: pop_rand -> u8, rate_rand -> u16
    (a comparison against a quantized uniform only flips when the
    threshold lands inside one quantization cell; p ~ 2^-9 / 2^-17).
    HBM traffic drops 47MB -> ~14MB/core.
  - pop_rand u8 is cast to bf16 in flight by SWDGE DMA (own queue, own
    engine stream); the [128,4096] compare runs on DVE in bf16 2x mode.
  - pop_W is fp8e4 (rhs of a bf16-lhsT matmul; random error ~2^-4.5 on
    0.02-scale weights perturbs pop rates by ~1e-3).
  - popcount over n and the weighted combine are accumulating
    scaled-identity matmuls on the PE; ACT casts psum -> u8
    (k = pop + 8*rate + 8*temporal + 4*sgn + 4 is an exact small int;
    host multiplies by w0/8 = 1/32).
  - phase: theta'-pi assembled in PSUM from bf16 split terms (phi1+phi2
    idents + 3 stacked S rows), folded to (-pi,pi] via ACT Sign + two
    split -pi idents, ACT Sin, ACT Sign(wv-0.5).
  - temporal one-hot via c_t = 1[st < t+0.5], temp = c_t - c_{t-1}.
  - All HWDGE DMA triggers live on the sync (SP) engine stream, ordered
    smalls -> rr -> W, so no compute engine stalls behind DMA queue
    backpressure; S rows are resident so waves never wait behind W.
"""

import os
import sys

for _p in ("/opt/trn_rl_repo", os.path.expanduser("~/.axon_site/_ro/trn_rl_repo")):
    if os.path.isdir(_p) and _p not in sys.path:
        sys.path.insert(0, _p)

import numpy as np
import ml_dtypes

import concourse.bacc as bacc
import concourse.mybir as mybir
import concourse.tile as tile
from concourse.bass_utils import run_bass_kernel_spmd

Alu = mybir.AluOpType
Act = mybir.ActivationFunctionType
F32 = mybir.dt.float32
BF16 = mybir.dt.bfloat16
FP8 = mybir.dt.float8e4
U8 = mybir.dt.uint8
U16 = mybir.dt.uint16

TWO_PI = 2.0 * np.pi

B, T, S, D, N = 4, 16, 256, 512, 8
NCORES = 8
NTOK = B * S                 # 1024 tokens
TOK = NTOK // NCORES         # 128 tokens per core (= partition dim)
DN = D * N                   # 4096
TD = T * D                   # 8192

BF = ml_dtypes.bfloat16
F8 = ml_dtypes.float8_e4m3


def _bf16_split(x, n):
    """Split float64 array into n bf16 terms summing (in f32) to ~x."""
    parts = []
    rem = x.astype(np.float64)
    for _ in range(n):
        p = rem.astype(BF)
        parts.append(p)
        rem = rem - p.astype(np.float64)
    return parts


def _build_program(coefs, has_bias, uniform):
    """coefs = (a_pop, a_r, a_c, a_s, bias0) baked into identity lhsTs."""
    from contextlib import ExitStack

    a_pop, a_r, a_c, a_s, bias0 = coefs
    nk = D // 128 + (1 if has_bias else 0)   # K-chunks of the pop matmul
    kdim = nk * 128

    nc = bacc.Bacc("TRN2", target_bir_lowering=False, debug=False,
                   num_devices=NCORES)

    emb = nc.dram_tensor("emb", [TOK, D], F32, kind="ExternalInput")
    embT = nc.dram_tensor("embT", [kdim, TOK], BF16, kind="ExternalInput")
    noise = nc.dram_tensor("noise", [TOK, D], F32, kind="ExternalInput")
    rr = nc.dram_tensor("rr", [TOK, TD], U16, kind="ExternalInput")
    pr = nc.dram_tensor("pr", [T, TOK, DN], U8, kind="ExternalInput")
    Wd = nc.dram_tensor("W", [kdim, DN], FP8, kind="ExternalInput")
    # idents blob: 8 scaled [128,128] idents + [3,128] ones rows
    idd = nc.dram_tensor("idents", [8 * 128 + 3, 128], BF16,
                         kind="ExternalInput")
    srd = nc.dram_tensor("srows", [3, TD], BF16, kind="ExternalInput")
    out_dt = U8 if uniform else BF16
    outd = nc.dram_tensor("out", [TOK, TD], out_dt, kind="ExternalOutput")

    with tile.TileContext(nc) as tc, ExitStack() as ctx:
        const = ctx.enter_context(tc.tile_pool(name="const", bufs=1))
        wp = ctx.enter_context(tc.tile_pool(name="wp", bufs=4))
        prp = ctx.enter_context(tc.tile_pool(name="prp", bufs=3))
        spkp = ctx.enter_context(tc.tile_pool(name="spkp", bufs=2))
        wvp = ctx.enter_context(tc.tile_pool(name="wvp", bufs=2))
        outp = ctx.enter_context(tc.tile_pool(name="outp", bufs=2))
        pm = ctx.enter_context(tc.tile_pool(name="pm", bufs=1, space="PSUM"))
        pw = ctx.enter_context(tc.tile_pool(name="pw", bufs=2, space="PSUM"))
        pc = ctx.enter_context(tc.tile_pool(name="pc", bufs=2, space="PSUM"))

        # ---- one-time loads, all on the SP queue, smalls first ----
        emb_sb = const.tile([TOK, D], F32)
        nc.sync.dma_start(emb_sb[:], emb[:])
        noise_sb = const.tile([TOK, D], F32)
        nc.sync.dma_start(noise_sb[:], noise[:])
        idt = []
        for i in range(8):
            it = const.tile([128, 128], BF16, tag=f"id{i}")
            nc.sync.dma_start(it[:], idd[i * 128:(i + 1) * 128, :])
            idt.append(it[:])
        I_pop, I_r, I_c, I_cm, I_s, I_1, I_p1, I_p2 = idt
        ones3t = const.tile([3, 128], BF16)
        nc.sync.dma_start(ones3t[:], idd[8 * 128:8 * 128 + 3, :])
        ones3 = ones3t[:]
        lhsT = []
        for k in range(nk):
            lt = const.tile([128, TOK], BF16, tag=f"lhsT{k}")
            nc.sync.dma_start(lt[:], embT[k * 128:(k + 1) * 128, :])
            lhsT.append(lt)
        srall = const.tile([3, TD], BF16)
        nc.sync.dma_start(srall[:], srd[:])
        rr_sb = const.tile([TOK, TD], U16)
        for j in range(4):
            nc.sync.dma_start(rr_sb[:, j * 4 * D:(j + 1) * 4 * D],
                              rr[:, j * 4 * D:(j + 1) * 4 * D])
        wts = []
        for k in range(nk):
            wt = wp.tile([128, DN], FP8, tag="w")
            nc.sync.dma_start(wt[:], Wd[k * 128:(k + 1) * 128, :])
            wts.append(wt)

        # ---- per-token precompute ----
        sig = const.tile([TOK, D], F32)
        nc.scalar.activation(sig[:], emb_sb[:], Act.Sigmoid)

        rates64 = const.tile([TOK, D], F32)
        tmp = const.tile([TOK, D], F32)
        nc.vector.tensor_scalar(tmp[:], sig[:], 0.9, 0.05, Alu.mult, Alu.add)
        nc.vector.scalar_tensor_tensor(tmp[:], noise_sb[:], 0.1, tmp[:],
                                       Alu.mult, Alu.add)
        nc.vector.tensor_scalar(tmp[:], tmp[:], 0.0, 1.0, Alu.max, Alu.min)
        nc.vector.tensor_scalar(rates64[:], tmp[:], 65536.0, None, Alu.mult)

        # st = floor(sig*(T-1)) via RNE(+-2^23) and round-down correction
        st = const.tile([TOK, D], F32)
        x15 = const.tile([TOK, D], F32)
        nc.vector.tensor_scalar(x15[:], sig[:], float(T - 1), None, Alu.mult)
        rnd = const.tile([TOK, D], F32)
        nc.vector.tensor_scalar(rnd[:], x15[:], 8388608.0, 8388608.0,
                                Alu.add, Alu.subtract)
        gtt = const.tile([TOK, D], F32)
        nc.vector.tensor_tensor(gtt[:], rnd[:], x15[:], Alu.is_gt)
        nc.vector.tensor_tensor(st[:], rnd[:], gtt[:], Alu.subtract)

        # phi split: phi = sig*2pi ; phi1 = bf16(phi) ; phi2 = bf16(phi-phi1)
        phi = const.tile([TOK, D], F32)
        nc.vector.tensor_scalar(phi[:], sig[:], TWO_PI, None, Alu.mult)
        phi1 = const.tile([TOK, D], BF16)
        nc.scalar.activation(phi1[:], phi[:], Act.Copy)
        phi2 = const.tile([TOK, D], BF16)
        nc.vector.tensor_tensor(phi2[:], phi[:], phi1[:], Alu.subtract)

        # ---- rate bits (u16 vs f32, 1x), follow rr chunk arrival ----
        rbit = const.tile([TOK, TD], BF16)
        for t in range(T):
            sl = slice(t * D, (t + 1) * D)
            nc.vector.tensor_tensor(rbit[:, sl], rr_sb[:, sl], rates64[:],
                                    Alu.is_lt)

        # ---- temporal cumulative bits: cz[:, (1+t)*D:] = 1[st < t+0.5] ----
        cz = const.tile([TOK, (T + 1) * D], BF16)
        nc.vector.memset(cz[:, 0:D], 0.0)
        for t in range(T):
            nc.vector.tensor_scalar(cz[:, (1 + t) * D:(2 + t) * D], st[:],
                                    t + 0.5, None, Alu.is_lt)

        # ---- pop linear: presp = emb @ W, sigmoid -> bf16, *256 in place ----
        prt256 = const.tile([TOK, DN], BF16)
        for h in range(2):
            ps = pm.tile([128, DN // 2], F32, tag="pm")
            for k in range(nk):
                for j in range(4):
                    sl = slice(j * 512, (j + 1) * 512)
                    nc.tensor.matmul(ps[:, sl], lhsT[k][:],
                                     wts[k][:, h * (DN // 2) + j * 512:
                                            h * (DN // 2) + (j + 1) * 512],
                                     start=(k == 0), stop=(k == nk - 1))
            nc.scalar.activation(prt256[:, h * (DN // 2):(h + 1) * (DN // 2)],
                                 ps[:], Act.Sigmoid)
        # *256 in place: bf16 exponent shift, exact
        nc.vector.tensor_scalar(prt256[:], prt256[:], 256.0, None, Alu.mult)

        # ---- waves: per t-step chunk of 512 cols ----
        # psum q = phi + S'_t  (S' = f32(t*f) - 2pi*k0 - pi, 3 bf16 rows)
        # sg = Sign(q); q += -pi_hi*sg + -pi_lo*sg  -> q in (-pi, pi]
        # wv = Sin(q); sgn_t = Sign(wv - 0.5)  in {-1, 0, 1}
        sgn = const.tile([TOK, TD], BF16)
        negh = const.tile([TOK, 1], F32)
        nc.vector.memset(negh[:], -0.5)

        def emit_wave(t):
            qs = pw.tile([128, D], F32, tag="pw")
            sl = slice(t * D, (t + 1) * D)
            nc.tensor.matmul(qs[:], I_1, phi1[:], start=True, stop=False)
            nc.tensor.matmul(qs[:], I_1, phi2[:], start=False, stop=False)
            nc.tensor.matmul(qs[:], ones3, srall[:, sl], start=False,
                             stop=True)
            sg = wvp.tile([TOK, D], BF16, tag="sg")
            nc.scalar.activation(sg[:], qs[:], Act.Sign)
            nc.tensor.matmul(qs[:], I_p1, sg[:], start=False, stop=False,
                             skip_group_check=True)
            nc.tensor.matmul(qs[:], I_p2, sg[:], start=False, stop=True,
                             skip_group_check=True)
            wv = wvp.tile([TOK, D], F32, tag="wv")
            nc.scalar.activation(wv[:], qs[:], Act.Sin)
            nc.scalar.activation(sgn[:, sl], wv[:], Act.Sign, bias=negh[:])

        for t in range(T):
            emit_wave(t)

        # ---- t-loop ----
        out_sb = None
        for t in range(T):
            if t % 4 == 0:
                out_sb = outp.tile([TOK, 4 * D], out_dt, tag="out")
            pt = prp.tile([TOK, DN], BF16, tag="pt")
            nc.gpsimd.dma_start(pt[:], pr[t])          # SWDGE u8 -> bf16 cast
            spk = spkp.tile([TOK, DN], BF16, tag="spk")
            nc.vector.tensor_tensor(spk[:], pt[:], prt256[:], Alu.is_lt)

            cs = pc.tile([128, D], F32, tag="pc")
            for n in range(8):
                nc.tensor.matmul(cs[:], I_pop, spk[:, n * D:(n + 1) * D],
                                 start=(n == 0), stop=False)
            sl = slice(t * D, (t + 1) * D)
            nc.tensor.matmul(cs[:], I_r, rbit[:, sl], start=False, stop=False)
            nc.tensor.matmul(cs[:], I_c, cz[:, (1 + t) * D:(2 + t) * D],
                             start=False, stop=False)
            nc.tensor.matmul(cs[:], I_cm, cz[:, t * D:(1 + t) * D],
                             start=False, stop=False)
            nc.tensor.matmul(cs[:], I_s, sgn[:, sl], start=False, stop=True)
            osl = slice((t % 4) * D, (t % 4 + 1) * D)
            nc.scalar.activation(out_sb[:, osl], cs[:], Act.Copy, bias=bias0,
                                 scale=1.0)
            if t % 4 == 3:
                nc.sync.dma_start(outd[:, (t - 3) * D:(t + 1) * D], out_sb[:])

    nc.compile()
    return nc


def _prepare_inputs(embeddings, pop_W, pop_b, freq_bands, enc_weights,
                    rate_noise, rate_rand, pop_rand):
    """Host-side sharding + layout/dtype transforms -> per-core in_maps."""
    e = np.exp(enc_weights.astype(np.float64)
               - enc_weights.astype(np.float64).max())
    w = e / e.sum()
    w0, w1, w2, w3 = [float(x) for x in w]
    uniform = abs(w1 - w0) < 1e-12 and abs(w3 - w0) < 1e-12 \
        and abs(w2 - w0) < 1e-12

    has_bias = bool(np.any(pop_b != 0))

    emb_f = np.ascontiguousarray(embeddings.reshape(NTOK, D))
    noise_f = np.ascontiguousarray(rate_noise.reshape(NTOK, D))
    # rate_rand [B,T,S,D] -> [BS, T*D] u16
    rr_f = rate_rand.transpose(0, 2, 1, 3).reshape(NTOK, TD)
    rr_u16 = np.minimum(np.floor(rr_f.astype(np.float64) * 65536.0),
                        65535).astype(np.uint16)
    # pop_rand [B,T,S,D,N] -> [BS, T, N*D] u8 (n-major feature axis)
    pr_f = pop_rand.transpose(0, 2, 1, 4, 3).reshape(NTOK, T, DN)
    pr_u8 = np.minimum(np.floor(pr_f.astype(np.float64) * 256.0),
                       255).astype(np.uint8)
    # pop_W columns to n-major: W2[k, n*D+d] = pop_W[k, d*N+n]
    W2 = pop_W.reshape(D, D, N).transpose(0, 2, 1).reshape(D, DN)
    if has_bias:
        b_nm = pop_b.reshape(D, N).T.reshape(1, DN)
        W2 = np.vstack([W2, b_nm, np.zeros((127, DN), np.float32)])
    W2 = np.ascontiguousarray(W2.astype(F8))

    # S rows: match jnp.linspace bit-exactly, tfc = f32(t*f) as jax does
    import jax
    import jax.numpy as jnp
    with jax.default_device(jax.devices("cpu")[0]):
        t_lin = np.asarray(jnp.linspace(0.0, TWO_PI, T)).astype(np.float64)
    tfc = (t_lin[:, None] * freq_bands.astype(np.float64)[None, :]
           ).astype(np.float32)                       # [T, D] f32 as jax
    k0 = np.round(tfc.astype(np.float64) / TWO_PI)
    Sp = tfc.astype(np.float64) - TWO_PI * k0 - np.pi   # in (-2pi, 0]
    s1, s2, s3 = _bf16_split(Sp.reshape(1, TD), 3)
    srows = np.ascontiguousarray(np.vstack([s1, s2, s3]))

    # coefficient idents
    if uniform:
        a_pop, a_r, a_c, a_s, bias0 = 1.0, 8.0, 8.0, 4.0, 4.0
    else:
        a_pop, a_r, a_c, a_s, bias0 = w2 / 8, w0, w1, w3 / 2, w3 / 2
    p1 = float(np.float64(np.pi).astype(BF))            # bf16(pi), exact rep
    p2 = float((np.float64(np.pi) - p1).astype(BF))
    I = np.eye(128, dtype=np.float64)
    blob = np.vstack([a_pop * I, a_r * I, a_c * I, -a_c * I, a_s * I,
                      1.0 * I, -p1 * I, -p2 * I,
                      np.ones((3, 128), np.float64)]).astype(BF)
    blob = np.ascontiguousarray(blob)

    in_maps = []
    for c in range(NCORES):
        t0, t1 = c * TOK, (c + 1) * TOK
        eT = emb_f[t0:t1].T
        if has_bias:
            eT = np.vstack([eT, np.ones((1, TOK), np.float32),
                            np.zeros((127, TOK), np.float32)])
        in_maps.append({
            "emb": emb_f[t0:t1],
            "embT": np.ascontiguousarray(eT.astype(BF)),
            "noise": noise_f[t0:t1],
            "rr": np.ascontiguousarray(rr_u16[t0:t1]),
            "pr": np.ascontiguousarray(pr_u8[t0:t1].transpose(1, 0, 2)),
            "W": W2,
            "idents": blob,
            "srows": srows,
        })
    return in_maps, (a_pop, a_r, a_c, a_s, bias0), has_bias, uniform, w0


_cache = {}


def kernel(embeddings, pop_W, pop_b, freq_bands, enc_weights,
           rate_noise, rate_rand, pop_rand, _want_trace=False):
    in_maps, coefs, has_bias, uniform, w0 = _prepare_inputs(
        embeddings, pop_W, pop_b, freq_bands, enc_weights,
        rate_noise, rate_rand, pop_rand)

    key = (coefs, has_bias, uniform)
    if key not in _cache:
        _cache[key] = _build_program(coefs, has_bias, uniform)
    nc = _cache[key]

    res = run_bass_kernel_spmd(nc, in_maps, core_ids=list(range(NCORES)),
                               trace=_want_trace)

    # out per core: [TOK, T*D] -> full [B, T, S, D]
    full = np.empty((NTOK, T, D), np.float32)
    for c in range(NCORES):
        o = res.results[c]["out"].astype(np.float32).reshape(TOK, T, D)
        full[c * TOK:(c + 1) * TOK] = o
    if uniform:
        full *= np.float32(w0 / 8.0)
    out = full.reshape(B, S, T, D).transpose(0, 2, 1, 3)
    out = np.ascontiguousarray(out)
    if _want_trace:
        kernel._last_trace = res
    return out


# revision 11
# speedup vs baseline: 1.1749x; 1.0123x over previous
"""Trainium2 Bass kernel for the BreakthroughSNN encoder problem (v4).

Per (b, t, s, d):
    out = w0*rate + w1*temporal + w2*pop + w3*phase
    rate    = 1[rate_rand < clip(sig*0.9+0.05+0.1*noise, 0, 1)]
    temporal= 1[floor(sig*(T-1)) == t]
    pop     = mean_n 1[pop_rand < sigmoid(emb @ pop_W + b)]
    phase   = 1[sin(freq_d*t_k + sig*2pi) > 0.5],  sig = sigmoid(emb)

Strategy:
  - Host quantizes the uniform randoms: pop_rand -> u8, rate_rand -> u16
    (a comparison against a quantized uniform only flips when the
    threshold lands inside one quantization cell; p ~ 2^-9 / 2^-17).
    HBM traffic drops 47MB -> ~14MB/core.
  - pop_rand u8 is cast to bf16 in flight by SWDGE DMA (own queue, own
    engine stream); the [128,4096] compare runs on DVE in bf16 2x mode.
  - pop_W is fp8e4 (rhs of a bf16-lhsT matmul; random error ~2^-4.5 on
    0.02-scale weights perturbs pop rates by ~1e-3).
  - popcount over n and the weighted combine are accumulating
    scaled-identity matmuls on the PE; ACT casts psum -> u8
    (k = pop + 8*rate + 8*temporal + 4*sgn + 4 is an exact small int;
    host multiplies by w0/8 = 1/32).
  - phase: theta'-pi assembled in PSUM from bf16 split terms (phi1+phi2
    idents + 3 stacked S rows), folded to (-pi,pi] via ACT Sign + two
    split -pi idents, ACT Sin, ACT Sign(wv-0.5).
  - temporal one-hot via c_t = 1[st < t+0.5], temp = c_t - c_{t-1}.
  - All HWDGE DMA triggers live on the sync (SP) engine stream, ordered
    smalls -> rr -> W, so no compute engine stalls behind DMA queue
    backpressure; S rows are resident so waves never wait behind W.
"""

import os
import sys

for _p in ("/opt/trn_rl_repo", os.path.expanduser("~/.axon_site/_ro/trn_rl_repo")):
    if os.path.isdir(_p) and _p not in sys.path:
        sys.path.insert(0, _p)

import numpy as np
import ml_dtypes

import concourse.bacc as bacc
import concourse.mybir as mybir
import concourse.tile as tile
from concourse.bass_utils import run_bass_kernel_spmd

Alu = mybir.AluOpType
Act = mybir.ActivationFunctionType
F32 = mybir.dt.float32
BF16 = mybir.dt.bfloat16
FP8 = mybir.dt.float8e4
U8 = mybir.dt.uint8
U16 = mybir.dt.uint16

TWO_PI = 2.0 * np.pi

B, T, S, D, N = 4, 16, 256, 512, 8
NCORES = 8
NTOK = B * S                 # 1024 tokens
TOK = NTOK // NCORES         # 128 tokens per core (= partition dim)
DN = D * N                   # 4096
TD = T * D                   # 8192

BF = ml_dtypes.bfloat16
F8 = ml_dtypes.float8_e4m3


def _bf16_split(x, n):
    """Split float64 array into n bf16 terms summing (in f32) to ~x."""
    parts = []
    rem = x.astype(np.float64)
    for _ in range(n):
        p = rem.astype(BF)
        parts.append(p)
        rem = rem - p.astype(np.float64)
    return parts


def _build_program(coefs, has_bias, uniform):
    """coefs = (a_pop, a_r, a_c, a_s, bias0) baked into identity lhsTs."""
    from contextlib import ExitStack

    a_pop, a_r, a_c, a_s, bias0 = coefs
    nk = D // 128 + (1 if has_bias else 0)   # K-chunks of the pop matmul
    kdim = nk * 128

    nc = bacc.Bacc("TRN2", target_bir_lowering=False, debug=False,
                   num_devices=NCORES)

    emb = nc.dram_tensor("emb", [TOK, D], F32, kind="ExternalInput")
    embT = nc.dram_tensor("embT", [kdim, TOK], BF16, kind="ExternalInput")
    noise = nc.dram_tensor("noise", [TOK, D], F32, kind="ExternalInput")
    rr = nc.dram_tensor("rr", [TOK, TD], U16, kind="ExternalInput")
    pr = nc.dram_tensor("pr", [T, TOK, DN], U8, kind="ExternalInput")
    Wd = nc.dram_tensor("W", [kdim, DN], BF16, kind="ExternalInput")
    # idents blob: 8 scaled [128,128] idents + [3,128] ones rows
    idd = nc.dram_tensor("idents", [8 * 128 + 3, 128], BF16,
                         kind="ExternalInput")
    srd = nc.dram_tensor("srows", [3, TD], BF16, kind="ExternalInput")
    out_dt = U8 if uniform else BF16
    outd = nc.dram_tensor("out", [TOK, TD], out_dt, kind="ExternalOutput")

    with tile.TileContext(nc) as tc, ExitStack() as ctx:
        const = ctx.enter_context(tc.tile_pool(name="const", bufs=1))
        wp = ctx.enter_context(tc.tile_pool(name="wp", bufs=4))
        prp = ctx.enter_context(tc.tile_pool(name="prp", bufs=4))
        spkp = ctx.enter_context(tc.tile_pool(name="spkp", bufs=2))
        wvp = ctx.enter_context(tc.tile_pool(name="wvp", bufs=2))
        outp = ctx.enter_context(tc.tile_pool(name="outp", bufs=2))
        pm = ctx.enter_context(tc.tile_pool(name="pm", bufs=1, space="PSUM"))
        pw = ctx.enter_context(tc.tile_pool(name="pw", bufs=2, space="PSUM"))
        pc = ctx.enter_context(tc.tile_pool(name="pc", bufs=2, space="PSUM"))

        # ---- one-time loads, all on the SP queue, smalls first ----
        emb_sb = const.tile([TOK, D], F32)
        nc.sync.dma_start(emb_sb[:], emb[:])
        noise_sb = const.tile([TOK, D], F32)
        nc.sync.dma_start(noise_sb[:], noise[:])
        idt = []
        for i in range(8):
            it = const.tile([128, 128], BF16, tag=f"id{i}")
            nc.sync.dma_start(it[:], idd[i * 128:(i + 1) * 128, :])
            idt.append(it[:])
        I_pop, I_r, I_c, I_cm, I_s, I_1, I_p1, I_p2 = idt
        ones3t = const.tile([3, 128], BF16)
        nc.sync.dma_start(ones3t[:], idd[8 * 128:8 * 128 + 3, :])
        ones3 = ones3t[:]
        lhsT = []
        for k in range(nk):
            lt = const.tile([128, TOK], BF16, tag=f"lhsT{k}")
            nc.sync.dma_start(lt[:], embT[k * 128:(k + 1) * 128, :])
            lhsT.append(lt)
        srall = const.tile([3, TD], BF16)
        nc.sync.dma_start(srall[:], srd[:])
        rr_sb = const.tile([TOK, TD], U16)
        for j in range(4):
            nc.sync.dma_start(rr_sb[:, j * 4 * D:(j + 1) * 4 * D],
                              rr[:, j * 4 * D:(j + 1) * 4 * D])
        wts = []
        for k in range(nk):
            wt = wp.tile([128, DN], BF16, tag="w")
            nc.sync.dma_start(wt[:], Wd[k * 128:(k + 1) * 128, :])
            wts.append(wt)

        # ---- per-token precompute ----
        sig = const.tile([TOK, D], F32)
        nc.scalar.activation(sig[:], emb_sb[:], Act.Sigmoid)

        rates64 = const.tile([TOK, D], F32)
        tmp = const.tile([TOK, D], F32)
        nc.vector.tensor_scalar(tmp[:], sig[:], 0.9, 0.05, Alu.mult, Alu.add)
        nc.vector.scalar_tensor_tensor(tmp[:], noise_sb[:], 0.1, tmp[:],
                                       Alu.mult, Alu.add)
        nc.vector.tensor_scalar(tmp[:], tmp[:], 0.0, 1.0, Alu.max, Alu.min)
        nc.vector.tensor_scalar(rates64[:], tmp[:], 65536.0, None, Alu.mult)

        # st = floor(sig*(T-1)) via RNE(+-2^23) and round-down correction
        st = const.tile([TOK, D], F32)
        x15 = const.tile([TOK, D], F32)
        nc.vector.tensor_scalar(x15[:], sig[:], float(T - 1), None, Alu.mult)
        rnd = const.tile([TOK, D], F32)
        nc.vector.tensor_scalar(rnd[:], x15[:], 8388608.0, 8388608.0,
                                Alu.add, Alu.subtract)
        gtt = const.tile([TOK, D], F32)
        nc.vector.tensor_tensor(gtt[:], rnd[:], x15[:], Alu.is_gt)
        nc.vector.tensor_tensor(st[:], rnd[:], gtt[:], Alu.subtract)

        # phi split: phi = sig*2pi ; phi1 = bf16(phi) ; phi2 = bf16(phi-phi1)
        phi = const.tile([TOK, D], F32)
        nc.vector.tensor_scalar(phi[:], sig[:], TWO_PI, None, Alu.mult)
        phi1 = const.tile([TOK, D], BF16)
        nc.scalar.activation(phi1[:], phi[:], Act.Copy)
        phi2 = const.tile([TOK, D], BF16)
        nc.vector.tensor_tensor(phi2[:], phi[:], phi1[:], Alu.subtract)

        # ---- rate bits (u16 vs f32, 1x), follow rr chunk arrival ----
        rbit = const.tile([TOK, TD], BF16)
        for t in range(T):
            sl = slice(t * D, (t + 1) * D)
            nc.vector.tensor_tensor(rbit[:, sl], rr_sb[:, sl], rates64[:],
                                    Alu.is_lt)

        # ---- temporal cumulative bits: cz[:, (1+t)*D:] = 1[st < t+0.5] ----
        cz = const.tile([TOK, (T + 1) * D], BF16)
        nc.vector.memset(cz[:, 0:D], 0.0)
        for t in range(T):
            nc.vector.tensor_scalar(cz[:, (1 + t) * D:(2 + t) * D], st[:],
                                    t + 0.5, None, Alu.is_lt)

        # ---- waves: per t-step chunk of 512 cols ----
        # psum q = phi + S'_t  (S' = f32(t*f) - 2pi*k0 - pi, 3 bf16 rows)
        # sg = Sign(q); q += -pi_hi*sg + -pi_lo*sg  -> q in (-pi, pi]
        # wv = Sin(q); sgn_t = Sign(wv - 0.5)  in {-1, 0, 1}
        sgn = const.tile([TOK, TD], BF16)
        negh = const.tile([TOK, 1], F32)
        nc.vector.memset(negh[:], -0.5)

        def emit_wave(t):
            qs = pw.tile([128, D], F32, tag="pw")
            sl = slice(t * D, (t + 1) * D)
            nc.tensor.matmul(qs[:], I_1, phi1[:], start=True, stop=False)
            nc.tensor.matmul(qs[:], I_1, phi2[:], start=False, stop=False)
            nc.tensor.matmul(qs[:], ones3, srall[:, sl], start=False,
                             stop=True)
            sg = wvp.tile([TOK, D], BF16, tag="sg")
            nc.scalar.activation(sg[:], qs[:], Act.Sign)
            nc.tensor.matmul(qs[:], I_p1, sg[:], start=False, stop=False,
                             skip_group_check=True)
            nc.tensor.matmul(qs[:], I_p2, sg[:], start=False, stop=True,
                             skip_group_check=True)
            wv = wvp.tile([TOK, D], F32, tag="wv")
            nc.scalar.activation(wv[:], qs[:], Act.Sin)
            nc.scalar.activation(sgn[:, sl], wv[:], Act.Sign, bias=negh[:])

        for t in range(T):
            emit_wave(t)

        # ---- pop linear: presp = emb @ W, sigmoid -> bf16, *256 ----
        prt256 = const.tile([TOK, DN], BF16)
        for h in range(2):
            ps = pm.tile([128, DN // 2], F32, tag="pm")
            for k in range(nk):
                for j in range(4):
                    sl = slice(j * 512, (j + 1) * 512)
                    nc.tensor.matmul(ps[:, sl], lhsT[k][:],
                                     wts[k][:, h * (DN // 2) + j * 512:
                                            h * (DN // 2) + (j + 1) * 512],
                                     start=(k == 0), stop=(k == nk - 1))
            hsl = slice(h * (DN // 2), (h + 1) * (DN // 2))
            nc.scalar.activation(prt256[:, hsl], ps[:], Act.Sigmoid)
            # *256 in place: bf16 exponent shift, exact
            nc.vector.tensor_scalar(prt256[:, hsl], prt256[:, hsl], 256.0,
                                    None, Alu.mult)

        # ---- t-loop ----
        HF = DN // 2
        out_sb = None
        for t in range(T):
            if t % 2 == 0:
                out_sb = outp.tile([TOK, 2 * D], out_dt, tag="out")
            pt = prp.tile([TOK, DN], BF16, tag="pt")
            nc.gpsimd.dma_start(pt[:], pr[t])          # SWDGE u8 -> bf16 cast
            spk = spkp.tile([TOK, DN], BF16, tag="spk")
            nc.vector.tensor_tensor(spk[:, 0:HF], pt[:, 0:HF],
                                    prt256[:, 0:HF], Alu.is_lt)
            nc.vector.tensor_tensor(spk[:, HF:DN], pt[:, HF:DN],
                                    prt256[:, HF:DN], Alu.is_lt)

            # extras first (their inputs are ready long before spk)
            cs = pc.tile([128, D], F32, tag="pc")
            sl = slice(t * D, (t + 1) * D)
            nc.tensor.matmul(cs[:], I_r, rbit[:, sl], start=True, stop=False)
            nc.tensor.matmul(cs[:], I_c, cz[:, (1 + t) * D:(2 + t) * D],
                             start=False, stop=False)
            nc.tensor.matmul(cs[:], I_cm, cz[:, t * D:(1 + t) * D],
                             start=False, stop=False)
            nc.tensor.matmul(cs[:], I_s, sgn[:, sl], start=False, stop=False)
            for n in range(8):
                nc.tensor.matmul(cs[:], I_pop, spk[:, n * D:(n + 1) * D],
                                 start=False, stop=(n == 7))
            osl = slice((t % 2) * D, (t % 2 + 1) * D)
            nc.scalar.activation(out_sb[:, osl], cs[:], Act.Copy, bias=bias0,
                                 scale=1.0)
            if t % 2 == 1:
                nc.sync.dma_start(outd[:, (t - 1) * D:(t + 1) * D], out_sb[:])

    nc.compile()
    return nc


def _prepare_inputs(embeddings, pop_W, pop_b, freq_bands, enc_weights,
                    rate_noise, rate_rand, pop_rand):
    """Host-side sharding + layout/dtype transforms -> per-core in_maps."""
    e = np.exp(enc_weights.astype(np.float64)
               - enc_weights.astype(np.float64).max())
    w = e / e.sum()
    w0, w1, w2, w3 = [float(x) for x in w]
    uniform = abs(w1 - w0) < 1e-12 and abs(w3 - w0) < 1e-12 \
        and abs(w2 - w0) < 1e-12

    has_bias = bool(np.any(pop_b != 0))

    emb_f = np.ascontiguousarray(embeddings.reshape(NTOK, D))
    noise_f = np.ascontiguousarray(rate_noise.reshape(NTOK, D))
    # rate_rand [B,T,S,D] -> [BS, T*D] u16
    rr_f = rate_rand.transpose(0, 2, 1, 3).reshape(NTOK, TD)
    rr_u16 = np.minimum(np.floor(rr_f.astype(np.float64) * 65536.0),
                        65535).astype(np.uint16)
    # pop_rand [B,T,S,D,N] -> [BS, T, N*D] u8 (n-major feature axis)
    pr_f = pop_rand.transpose(0, 2, 1, 4, 3).reshape(NTOK, T, DN)
    pr_u8 = np.minimum(np.floor(pr_f.astype(np.float64) * 256.0),
                       255).astype(np.uint8)
    # pop_W columns to n-major: W2[k, n*D+d] = pop_W[k, d*N+n]
    W2 = pop_W.reshape(D, D, N).transpose(0, 2, 1).reshape(D, DN)
    if has_bias:
        b_nm = pop_b.reshape(D, N).T.reshape(1, DN)
        W2 = np.vstack([W2, b_nm, np.zeros((127, DN), np.float32)])
    W2 = np.ascontiguousarray(W2.astype(BF))

    # S rows: match jnp.linspace bit-exactly, tfc = f32(t*f) as jax does
    import jax
    import jax.numpy as jnp
    with jax.default_device(jax.devices("cpu")[0]):
        t_lin = np.asarray(jnp.linspace(0.0, TWO_PI, T)).astype(np.float64)
    tfc = (t_lin[:, None] * freq_bands.astype(np.float64)[None, :]
           ).astype(np.float32)                       # [T, D] f32 as jax
    k0 = np.round(tfc.astype(np.float64) / TWO_PI)
    Sp = tfc.astype(np.float64) - TWO_PI * k0 - np.pi   # in (-2pi, 0]
    s1, s2, s3 = _bf16_split(Sp.reshape(1, TD), 3)
    srows = np.ascontiguousarray(np.vstack([s1, s2, s3]))

    # coefficient idents
    if uniform:
        a_pop, a_r, a_c, a_s, bias0 = 1.0, 8.0, 8.0, 4.0, 4.0
    else:
        a_pop, a_r, a_c, a_s, bias0 = w2 / 8, w0, w1, w3 / 2, w3 / 2
    p1 = float(np.float64(np.pi).astype(BF))            # bf16(pi), exact rep
    p2 = float((np.float64(np.pi) - p1).astype(BF))
    I = np.eye(128, dtype=np.float64)
    blob = np.vstack([a_pop * I, a_r * I, a_c * I, -a_c * I, a_s * I,
                      1.0 * I, -p1 * I, -p2 * I,
                      np.ones((3, 128), np.float64)]).astype(BF)
    blob = np.ascontiguousarray(blob)

    in_maps = []
    for c in range(NCORES):
        t0, t1 = c * TOK, (c + 1) * TOK
        eT = emb_f[t0:t1].T
        if has_bias:
            eT = np.vstack([eT, np.ones((1, TOK), np.float32),
                            np.zeros((127, TOK), np.float32)])
        in_maps.append({
            "emb": emb_f[t0:t1],
            "embT": np.ascontiguousarray(eT.astype(BF)),
            "noise": noise_f[t0:t1],
            "rr": np.ascontiguousarray(rr_u16[t0:t1]),
            "pr": np.ascontiguousarray(pr_u8[t0:t1].transpose(1, 0, 2)),
            "W": W2,
            "idents": blob,
            "srows": srows,
        })
    return in_maps, (a_pop, a_r, a_c, a_s, bias0), has_bias, uniform, w0


_cache = {}


def kernel(embeddings, pop_W, pop_b, freq_bands, enc_weights,
           rate_noise, rate_rand, pop_rand, _want_trace=False):
    in_maps, coefs, has_bias, uniform, w0 = _prepare_inputs(
        embeddings, pop_W, pop_b, freq_bands, enc_weights,
        rate_noise, rate_rand, pop_rand)

    key = (coefs, has_bias, uniform)
    if key not in _cache:
        _cache[key] = _build_program(coefs, has_bias, uniform)
    nc = _cache[key]

    res = run_bass_kernel_spmd(nc, in_maps, core_ids=list(range(NCORES)),
                               trace=_want_trace)

    # out per core: [TOK, T*D] -> full [B, T, S, D]
    full = np.empty((NTOK, T, D), np.float32)
    for c in range(NCORES):
        o = res.results[c]["out"].astype(np.float32).reshape(TOK, T, D)
        full[c * TOK:(c + 1) * TOK] = o
    if uniform:
        full *= np.float32(w0 / 8.0)
    out = full.reshape(B, S, T, D).transpose(0, 2, 1, 3)
    out = np.ascontiguousarray(out)
    if _want_trace:
        kernel._last_trace = res
    return out
